# revision 12
# baseline (speedup 1.0000x reference)
"""DenseEnergyLoss Trainium2 kernel — Kronecker-eigen x polynomial factorization.

loss = WEIGHT * (-1/n) * sum_k A'_k^T G B'_k,   G[i,j] = exp(f_i . f_j)

with f = (x/50, y/50, rgb/15) per downsampled pixel (P = 64*64 = 4096),
A' = seg_r * gate * e,  B' = seg_r * e,  e = exp(-0.5|f|^2).

G factors exactly as  exp((x x' + y y')/2500) * exp(rgb.rgb'/225):
  * the xy part is a CONSTANT Kronecker kernel M ⊗ M with M[a,b] =
    exp(ab/2500) (64x64).  M's spectrum decays ~6 orders in 5 modes, so
    M ≈ Q_r Λ_r Q_r^T with r = 4 is far below the bf16 noise floor.
  * the rgb part has |s| = |rgb.rgb'|/225 <= ~0.2 (typ. ~0.01), so
    exp(s) ≈ 1 + s pointwise to ~1e-3; the resulting 4-term feature map
    is psi = (1, r/15, g/15, b/15).

Then G ≈ K K^T ∘ (Psi Psi^T) with K = (Q√Λ ⊗ Q√Λ) [P, 16] constant and
loss_img = Σ_{k,α,ij} (K^T (A'_k ∘ ψ_α))_ij (K^T (B'_k ∘ ψ_α))_ij.

Per core (8 = 4 images x {A-side, B-side}): the ψ_0 = 1 part needs no
multiply — its 21 columns are the side matrix itself, fed to the PE
directly.  Only ψ_1..3 (63 cols/block) are built by Vector-engine
broadcast multiplies.  Pixel blocks 0-15 accumulate at PE column
position 0, blocks 16-31 at position 32, so the first half's PSUM
stripe is copied out and DMA'd while the second half still computes.
Inputs arrive as two column-split DMAs per queue (sync: per-block
interleaved [A'|psi], scalar: K), first-needed half first.  Host sums
the two [16, 84] partial outputs per core and takes the A.B dot per
image.  End-to-end rel err vs the exact reference ~ 6e-5 (bf16
rounding dominated).
"""

import numpy as np
import ml_dtypes

WEIGHT = 1e-07
SIGMA_RGB = 15.0
SIGMA_XY_EFF = 50.0  # SIGMA_XY * SCALE
IGNORE_LABEL = 255

N_IMG = 4
K_CLS = 21
H_DS = 64
P = H_DS * H_DS  # 4096
R_EIG = 4
R2 = R_EIG * R_EIG  # 16
N_MONO = 4  # psi = (1, r, g, b)
NBLK = 32  # pixel blocks of 128
AUG_BLKS = 4  # blocks per aug op
W_AUG = N_MONO * K_CLS  # 84
W_AUG3 = (N_MONO - 1) * K_CLS  # 63 (psi_1..3 part)
C_BLK = K_CLS + N_MONO  # 25: per-block [ab | psi] interleave
C_INAB = NBLK * C_BLK  # 800

BF16 = ml_dtypes.bfloat16

_CACHE = {}


def _build_program():
    import concourse.bacc as bacc
    import concourse.tile as tile
    from concourse import mybir

    f32 = mybir.dt.float32
    bf16 = mybir.dt.bfloat16

    nc = bacc.Bacc("TRN2", target_bir_lowering=False, debug=False)

    inab_d = nc.dram_tensor("inab", [128, C_INAB], bf16, kind="ExternalInput")
    kc_d = nc.dram_tensor("kc", [128, NBLK * R2], bf16, kind="ExternalInput")
    ua_d = nc.dram_tensor("u_a", [R2, W_AUG], f32, kind="ExternalOutput")
    ub_d = nc.dram_tensor("u_b", [R2, W_AUG], f32, kind="ExternalOutput")

    with tile.TileContext(nc) as tc:
        with (
            tc.tile_pool(name="const", bufs=1) as cpool,
            tc.tile_pool(name="aug", bufs=4) as augpool,
            tc.tile_pool(name="ps", bufs=1, space="PSUM") as pspool,
            tc.tile_pool(name="outp", bufs=1) as opool,
        ):
            inab = cpool.tile([128, C_INAB], bf16, tag="inab")
            kc = cpool.tile([128, NBLK * R2], bf16, tag="kc")
            half_ab = C_INAB // 2
            # Input DMAs are packet-bound (one packet per partition row), so
            # split each tensor across the two HWDGE queues by partition
            # halves; first-needed columns first.
            nc.sync.dma_start(inab[0:64, 0:half_ab], inab_d[0:64, 0:half_ab])
            nc.scalar.dma_start(
                inab[64:128, 0:half_ab], inab_d[64:128, 0:half_ab]
            )
            nc.sync.dma_start(kc[0:64, :], kc_d[0:64, :])
            nc.scalar.dma_start(kc[64:128, :], kc_d[64:128, :])
            nc.sync.dma_start(inab[0:64, half_ab:], inab_d[0:64, half_ab:])
            nc.scalar.dma_start(
                inab[64:128, half_ab:], inab_d[64:128, half_ab:]
            )

            osb = opool.tile([128, W_AUG], f32, tag="o")
            ps = pspool.tile([128, W_AUG], f32, tag="ps")

            inab3 = inab[:].rearrange("p (b c) -> p b c", c=C_BLK)
            n_tiles = NBLK // AUG_BLKS
            for t in range(n_tiles):
                grp = t // (n_tiles // 2)  # blocks 0-15 -> 0, 16-31 -> 1
                aug = augpool.tile([128, AUG_BLKS * W_AUG3], bf16, tag="aug")
                a4 = inab3[:, t * AUG_BLKS : (t + 1) * AUG_BLKS, 0:K_CLS]
                p3 = inab3[
                    :, t * AUG_BLKS : (t + 1) * AUG_BLKS, K_CLS + 1 : C_BLK
                ]
                o4 = aug[:].rearrange(
                    "p (b m k) -> p b m k", m=N_MONO - 1, k=K_CLS
                )
                nc.vector.tensor_tensor(
                    o4,
                    a4.unsqueeze(2).broadcast_to(
                        [128, AUG_BLKS, N_MONO - 1, K_CLS]
                    ),
                    p3.unsqueeze(3).broadcast_to(
                        [128, AUG_BLKS, N_MONO - 1, K_CLS]
                    ),
                    mybir.AluOpType.mult,
                )
                for j in range(AUG_BLKS):
                    blk = AUG_BLKS * t + j
                    first = blk % (NBLK // 2) == 0
                    last = blk % (NBLK // 2) == NBLK // 2 - 1
                    # start/stop bracket the whole accumulation group at
                    # this tile position: start clears has_written for the
                    # entire position, so only the group's first MM may
                    # carry it (and only the last carries stop).
                    nc.tensor.matmul(
                        ps[32 * grp : 32 * grp + R2, 0:K_CLS],
                        kc[:, blk * R2 : (blk + 1) * R2],
                        inab3[:, blk, 0:K_CLS],
                        start=first,
                        stop=False,
                        tile_position=(0, 32 * grp),
                        skip_group_check=True,
                    )
                    nc.tensor.matmul(
                        ps[32 * grp : 32 * grp + R2, K_CLS:W_AUG],
                        kc[:, blk * R2 : (blk + 1) * R2],
                        aug[:, j * W_AUG3 : (j + 1) * W_AUG3],
                        start=False,
                        stop=last,
                        tile_position=(0, 32 * grp),
                        skip_group_check=True,
                    )
            nc.vector.tensor_copy(osb[0:R2, :], ps[0:R2, :])
            nc.sync.dma_start(ua_d[:], osb[0:R2, :], single_packet=True)
            nc.vector.tensor_copy(osb[32 : 32 + R2, :], ps[32 : 32 + R2, :])
            nc.scalar.dma_start(ub_d[:], osb[32 : 32 + R2, :], single_packet=True)

    nc.compile()
    return nc


def _host_prep(images, segmentations, ROIs, seg_label):
    """Returns the 8 per-core input dicts. Core 2i -> A-side of image i,
    core 2i+1 -> B-side."""
    imgs = images[:, :, ::2, ::2].astype(np.float64)  # [N,3,64,64]
    segs = (
        segmentations.astype(np.float64)
        .reshape(N_IMG, K_CLS, H_DS, 2, H_DS, 2)
        .mean(axis=(3, 5))
    )
    rois = ROIs[:, ::2, ::2].astype(np.float64)
    lbl = seg_label[:, 0, ::2, ::2]
    unlabel = lbl == IGNORE_LABEL

    seg_max = segs.max(axis=1)
    gate = np.where(unlabel, 1.0, rois - seg_max)
    gate = np.maximum(gate, 0.0)  # [N,64,64]
    seg_r = segs * rois[:, None]  # [N,21,64,64]

    yy, xx = np.meshgrid(
        np.arange(H_DS, dtype=np.float64),
        np.arange(H_DS, dtype=np.float64),
        indexing="ij",
    )
    sq_xy = ((xx / SIGMA_XY_EFF) ** 2 + (yy / SIGMA_XY_EFF) ** 2).reshape(P)
    u = imgs.reshape(N_IMG, 3, P) / SIGMA_RGB  # [N,3,P]
    e = np.exp(-0.5 * (sq_xy[None, :] + (u * u).sum(axis=1)))  # [N,P]
    Bp = seg_r.reshape(N_IMG, K_CLS, P) * e[:, None, :]
    Ap = Bp * gate.reshape(N_IMG, P)[:, None, :]

    # constant Kronecker factor K = (Q sqrt(L)) x (Q sqrt(L)), top R_EIG
    ax = np.arange(H_DS, dtype=np.float64) / SIGMA_XY_EFF
    M = np.exp(np.outer(ax, ax))
    w_eig, Q = np.linalg.eigh(M)
    lam = w_eig[::-1][:R_EIG]
    Qr = Q[:, ::-1][:, :R_EIG]
    Ky = Qr * np.sqrt(lam)[None, :]  # [64, R]
    Kfull = np.einsum("yi,xj->yxij", Ky, Ky).reshape(P, R2)

    def blockmajor(x):  # [P, C] f64 -> [128, 32*C] bf16
        c = x.shape[1]
        return np.ascontiguousarray(
            x.reshape(NBLK, 128, c).transpose(1, 0, 2).reshape(128, NBLK * c)
        ).astype(BF16)

    kc_bm = blockmajor(Kfull)

    in_maps = []
    for img in range(N_IMG):
        psi = np.concatenate([np.ones((1, P)), u[img]], axis=0).T  # [P, 4]
        for side_mat in (Ap[img], Bp[img]):  # A side then B side
            inab = np.concatenate([side_mat.T, psi], axis=1)  # [P, 25]
            in_maps.append(
                {"inab": blockmajor(inab), "kc": kc_bm}
            )
    return in_maps


def _get_program():
    if "nc" not in _CACHE:
        _CACHE["nc"] = _build_program()
    return _CACHE["nc"]


def _install_profile_hook():
    """Best-effort registration of the axon NTFF profile hook so that
    trace=True works (used by test harness, not the plain kernel path)."""
    import sys
    import types

    if "antenv.axon_hooks" in sys.modules:
        return
    try:
        from trn_agent_boot.trn_boot import _ntff_profile_via_ctypes

        hook = _ntff_profile_via_ctypes("/opt/axon/libaxon_pjrt.so")
        mod = types.ModuleType("antenv.axon_hooks")
        mod.get_axon_ntff_profile_hook = lambda: hook
        sys.modules["antenv.axon_hooks"] = mod
    except Exception:
        pass


def kernel(images, segmentations, ROIs, seg_label, _trace=False, _tmpdir=None):
    from concourse import bass_utils

    in_maps = _host_prep(images, segmentations, ROIs, seg_label)
    nc = _get_program()
    if _trace:
        _install_profile_hook()
        bass_utils.upload_artifacts = lambda tmpdir: f"local:{tmpdir}"
    res = bass_utils.run_bass_kernel_spmd(
        nc, in_maps, list(range(8)), trace=_trace, tmpdir=_tmpdir
    )
    total = 0.0
    us = []
    for r in res.results:
        us.append(
            r["u_a"].astype(np.float64) + r["u_b"].astype(np.float64)
        )  # [16, 84]
    for img in range(N_IMG):
        total += np.sum(us[2 * img] * us[2 * img + 1])
    loss = np.float32(-WEIGHT / N_IMG * total)
    if _trace:
        return np.array([loss], np.float32), res
    return np.array([loss], np.float32)


# revision 15
# speedup vs baseline: 1.0315x; 1.0315x over previous
"""DenseEnergyLoss Trainium2 kernel — Kronecker-eigen x polynomial factorization.

loss = WEIGHT * (-1/n) * sum_k A'_k^T G B'_k,   G[i,j] = exp(f_i . f_j)

with f = (x/50, y/50, rgb/15) per downsampled pixel (P = 64*64 = 4096),
A' = seg_r * gate * e,  B' = seg_r * e,  e = exp(-0.5|f|^2).

G factors exactly as  exp((x x' + y y')/2500) * exp(rgb.rgb'/225):
  * the xy part is a CONSTANT Kronecker kernel M ⊗ M with M[a,b] =
    exp(ab/2500) (64x64).  M's spectrum decays ~6 orders in 5 modes, so
    M ≈ Q_r Λ_r Q_r^T with r = 4 is far below the bf16 noise floor.
  * the rgb part has |s| = |rgb.rgb'|/225 <= ~0.2 (typ. ~0.01), so
    exp(s) ≈ 1 + s pointwise to ~1e-3; the resulting 4-term feature map
    is psi = (1, r/15, g/15, b/15).

Then G ≈ K K^T ∘ (Psi Psi^T) with K = (Q√Λ ⊗ Q√Λ) [P, 16] constant and
loss_img = Σ_{k,α,ij} (K^T (A'_k ∘ ψ_α))_ij (K^T (B'_k ∘ ψ_α))_ij.

Per core (8 = 4 images x {A-side, B-side}): the ψ_0 = 1 part needs no
multiply — its 21 columns are the side matrix itself, fed to the PE
directly.  Only ψ_1..3 (63 cols/block) are built by Vector-engine
broadcast multiplies.  Pixel blocks 0-15 accumulate at PE column
position 0, blocks 16-31 at position 32, so the first half's PSUM
stripe is copied out and DMA'd while the second half still computes.
Inputs arrive as two column-split DMAs per queue (sync: per-block
interleaved [A'|psi], scalar: K), first-needed half first.  Host sums
the two [16, 84] partial outputs per core and takes the A.B dot per
image.  End-to-end rel err vs the exact reference ~ 6e-5 (bf16
rounding dominated).
"""

import numpy as np
import ml_dtypes

WEIGHT = 1e-07
SIGMA_RGB = 15.0
SIGMA_XY_EFF = 50.0  # SIGMA_XY * SCALE
IGNORE_LABEL = 255

N_IMG = 4
K_CLS = 21
H_DS = 64
P = H_DS * H_DS  # 4096
R_EIG = 4
R2 = R_EIG * R_EIG  # 16
N_MONO = 4  # psi = (1, r, g, b)
NBLK = 32  # pixel blocks of 128
AUG_BLKS = 8  # blocks per aug op
W_AUG = N_MONO * K_CLS  # 84
W_AUG3 = (N_MONO - 1) * K_CLS  # 63 (psi_1..3 part)
C_BLK = K_CLS + N_MONO  # 25: per-block [ab | psi] interleave
C_INAB = NBLK * C_BLK  # 800

BF16 = ml_dtypes.bfloat16

_CACHE = {}


def _build_program():
    import concourse.bacc as bacc
    import concourse.tile as tile
    from concourse import mybir

    f32 = mybir.dt.float32
    bf16 = mybir.dt.bfloat16

    nc = bacc.Bacc("TRN2", target_bir_lowering=False, debug=False)

    inab_d = nc.dram_tensor("inab", [128, C_INAB], bf16, kind="ExternalInput")
    kc_d = nc.dram_tensor("kc", [128, NBLK * R2], bf16, kind="ExternalInput")
    ua_d = nc.dram_tensor("u_a", [R2, W_AUG], f32, kind="ExternalOutput")
    ub_d = nc.dram_tensor("u_b", [R2, W_AUG], f32, kind="ExternalOutput")

    with tile.TileContext(nc) as tc:
        with (
            tc.tile_pool(name="const", bufs=1) as cpool,
            tc.tile_pool(name="aug", bufs=4) as augpool,
            tc.tile_pool(name="ps", bufs=1, space="PSUM") as pspool,
            tc.tile_pool(name="outp", bufs=1) as opool,
        ):
            inab = cpool.tile([128, C_INAB], bf16, tag="inab")
            kc = cpool.tile([128, NBLK * R2], bf16, tag="kc")
            half_ab = 20 * C_BLK  # first 20 blocks; rest arrive mid-stream
            half_kc = NBLK * R2 // 2
            # Per-DMA completion latency is ~2us regardless of size, so keep
            # the critical first chunk as a single DMA per queue.
            nc.sync.dma_start(inab[:, 0:half_ab], inab_d[:, 0:half_ab])
            nc.scalar.dma_start(kc[:, 0:half_kc], kc_d[:, 0:half_kc])
            nc.sync.dma_start(inab[:, half_ab:], inab_d[:, half_ab:])
            nc.scalar.dma_start(kc[:, half_kc:], kc_d[:, half_kc:])

            osb = opool.tile([128, W_AUG], f32, tag="o")
            ps = pspool.tile([128, W_AUG], f32, tag="ps")

            inab3 = inab[:].rearrange("p (b c) -> p b c", c=C_BLK)
            n_tiles = NBLK // AUG_BLKS
            for t in range(n_tiles):
                grp = t // (n_tiles // 2)  # blocks 0-15 -> 0, 16-31 -> 1
                aug = augpool.tile([128, AUG_BLKS * W_AUG3], bf16, tag="aug")
                a4 = inab3[:, t * AUG_BLKS : (t + 1) * AUG_BLKS, 0:K_CLS]
                p3 = inab3[
                    :, t * AUG_BLKS : (t + 1) * AUG_BLKS, K_CLS + 1 : C_BLK
                ]
                o4 = aug[:].rearrange(
                    "p (b m k) -> p b m k", m=N_MONO - 1, k=K_CLS
                )
                nc.vector.tensor_tensor(
                    o4,
                    a4.unsqueeze(2).broadcast_to(
                        [128, AUG_BLKS, N_MONO - 1, K_CLS]
                    ),
                    p3.unsqueeze(3).broadcast_to(
                        [128, AUG_BLKS, N_MONO - 1, K_CLS]
                    ),
                    mybir.AluOpType.mult,
                )
                for j in range(AUG_BLKS):
                    blk = AUG_BLKS * t + j
                    first = blk % (NBLK // 2) == 0
                    last = blk % (NBLK // 2) == NBLK // 2 - 1
                    # start/stop bracket the whole accumulation group at
                    # this tile position: start clears has_written for the
                    # entire position, so only the group's first MM may
                    # carry it (and only the last carries stop).
                    nc.tensor.matmul(
                        ps[32 * grp : 32 * grp + R2, 0:K_CLS],
                        kc[:, blk * R2 : (blk + 1) * R2],
                        inab3[:, blk, 0:K_CLS],
                        start=first,
                        stop=False,
                        tile_position=(0, 32 * grp),
                        skip_group_check=True,
                    )
                    nc.tensor.matmul(
                        ps[32 * grp : 32 * grp + R2, K_CLS:W_AUG],
                        kc[:, blk * R2 : (blk + 1) * R2],
                        aug[:, j * W_AUG3 : (j + 1) * W_AUG3],
                        start=False,
                        stop=last,
                        tile_position=(0, 32 * grp),
                        skip_group_check=True,
                    )
            nc.vector.tensor_copy(osb[0:R2, :], ps[0:R2, :])
            nc.sync.dma_start(ua_d[:], osb[0:R2, :], single_packet=True)
            nc.vector.tensor_copy(osb[32 : 32 + R2, :], ps[32 : 32 + R2, :])
            nc.scalar.dma_start(ub_d[:], osb[32 : 32 + R2, :], single_packet=True)

    nc.compile()
    return nc


def _host_prep(images, segmentations, ROIs, seg_label):
    """Returns the 8 per-core input dicts. Core 2i -> A-side of image i,
    core 2i+1 -> B-side."""
    imgs = images[:, :, ::2, ::2].astype(np.float64)  # [N,3,64,64]
    segs = (
        segmentations.astype(np.float64)
        .reshape(N_IMG, K_CLS, H_DS, 2, H_DS, 2)
        .mean(axis=(3, 5))
    )
    rois = ROIs[:, ::2, ::2].astype(np.float64)
    lbl = seg_label[:, 0, ::2, ::2]
    unlabel = lbl == IGNORE_LABEL

    seg_max = segs.max(axis=1)
    gate = np.where(unlabel, 1.0, rois - seg_max)
    gate = np.maximum(gate, 0.0)  # [N,64,64]
    seg_r = segs * rois[:, None]  # [N,21,64,64]

    yy, xx = np.meshgrid(
        np.arange(H_DS, dtype=np.float64),
        np.arange(H_DS, dtype=np.float64),
        indexing="ij",
    )
    sq_xy = ((xx / SIGMA_XY_EFF) ** 2 + (yy / SIGMA_XY_EFF) ** 2).reshape(P)
    u = imgs.reshape(N_IMG, 3, P) / SIGMA_RGB  # [N,3,P]
    e = np.exp(-0.5 * (sq_xy[None, :] + (u * u).sum(axis=1)))  # [N,P]
    Bp = seg_r.reshape(N_IMG, K_CLS, P) * e[:, None, :]
    Ap = Bp * gate.reshape(N_IMG, P)[:, None, :]

    # constant Kronecker factor K = (Q sqrt(L)) x (Q sqrt(L)), top R_EIG
    ax = np.arange(H_DS, dtype=np.float64) / SIGMA_XY_EFF
    M = np.exp(np.outer(ax, ax))
    w_eig, Q = np.linalg.eigh(M)
    lam = w_eig[::-1][:R_EIG]
    Qr = Q[:, ::-1][:, :R_EIG]
    Ky = Qr * np.sqrt(lam)[None, :]  # [64, R]
    Kfull = np.einsum("yi,xj->yxij", Ky, Ky).reshape(P, R2)

    def blockmajor(x):  # [P, C] f64 -> [128, 32*C] bf16
        c = x.shape[1]
        return np.ascontiguousarray(
            x.reshape(NBLK, 128, c).transpose(1, 0, 2).reshape(128, NBLK * c)
        ).astype(BF16)

    kc_bm = blockmajor(Kfull)

    in_maps = []
    for img in range(N_IMG):
        psi = np.concatenate([np.ones((1, P)), u[img]], axis=0).T  # [P, 4]
        for side_mat in (Ap[img], Bp[img]):  # A side then B side
            inab = np.concatenate([side_mat.T, psi], axis=1)  # [P, 25]
            in_maps.append(
                {"inab": blockmajor(inab), "kc": kc_bm}
            )
    return in_maps


def _get_program():
    if "nc" not in _CACHE:
        _CACHE["nc"] = _build_program()
    return _CACHE["nc"]


def _install_profile_hook():
    """Best-effort registration of the axon NTFF profile hook so that
    trace=True works (used by test harness, not the plain kernel path)."""
    import sys
    import types

    if "antenv.axon_hooks" in sys.modules:
        return
    try:
        from trn_agent_boot.trn_boot import _ntff_profile_via_ctypes

        hook = _ntff_profile_via_ctypes("/opt/axon/libaxon_pjrt.so")
        mod = types.ModuleType("antenv.axon_hooks")
        mod.get_axon_ntff_profile_hook = lambda: hook
        sys.modules["antenv.axon_hooks"] = mod
    except Exception:
        pass


def kernel(images, segmentations, ROIs, seg_label, _trace=False, _tmpdir=None):
    from concourse import bass_utils

    in_maps = _host_prep(images, segmentations, ROIs, seg_label)
    nc = _get_program()
    if _trace:
        _install_profile_hook()
        bass_utils.upload_artifacts = lambda tmpdir: f"local:{tmpdir}"
    res = bass_utils.run_bass_kernel_spmd(
        nc, in_maps, list(range(8)), trace=_trace, tmpdir=_tmpdir
    )
    total = 0.0
    us = []
    for r in res.results:
        us.append(
            r["u_a"].astype(np.float64) + r["u_b"].astype(np.float64)
        )  # [16, 84]
    for img in range(N_IMG):
        total += np.sum(us[2 * img] * us[2 * img + 1])
    loss = np.float32(-WEIGHT / N_IMG * total)
    if _trace:
        return np.array([loss], np.float32), res
    return np.array([loss], np.float32)


# revision 16
# speedup vs baseline: 1.0494x; 1.0174x over previous
"""DenseEnergyLoss Trainium2 kernel — Kronecker-eigen x polynomial factorization.

loss = WEIGHT * (-1/n) * sum_k A'_k^T G B'_k,   G[i,j] = exp(f_i . f_j)

with f = (x/50, y/50, rgb/15) per downsampled pixel (P = 64*64 = 4096),
A' = seg_r * gate * e,  B' = seg_r * e,  e = exp(-0.5|f|^2).

G factors exactly as  exp((x x' + y y')/2500) * exp(rgb.rgb'/225):
  * the xy part is a CONSTANT Kronecker kernel M ⊗ M with M[a,b] =
    exp(ab/2500) (64x64).  M's spectrum decays ~6 orders in 5 modes, so
    M ≈ Q_r Λ_r Q_r^T with r = 4 is far below the bf16 noise floor.
  * the rgb part has |s| = |rgb.rgb'|/225 <= ~0.2 (typ. ~0.01), so
    exp(s) ≈ 1 + s pointwise to ~1e-3; the resulting 4-term feature map
    is psi = (1, r/15, g/15, b/15).

Then G ≈ K K^T ∘ (Psi Psi^T) with K = (Q√Λ ⊗ Q√Λ) [P, 16] constant and
loss_img = Σ_{k,α,ij} (K^T (A'_k ∘ ψ_α))_ij (K^T (B'_k ∘ ψ_α))_ij.

Per core (8 = 4 images x {A-side, B-side}): the ψ_0 = 1 part needs no
multiply — its 21 columns are the side matrix itself, fed to the PE
directly.  Only ψ_1..3 (63 cols/block) are built by Vector-engine
broadcast multiplies.  Pixel blocks 0-15 accumulate at PE column
position 0, blocks 16-31 at position 32, so the first half's PSUM
stripe is copied out and DMA'd while the second half still computes.
Inputs arrive as two column-split DMAs per queue (sync: per-block
interleaved [A'|psi], scalar: K), first-needed half first.  Host sums
the two [16, 84] partial outputs per core and takes the A.B dot per
image.  End-to-end rel err vs the exact reference ~ 6e-5 (bf16
rounding dominated).
"""

import numpy as np
import ml_dtypes

WEIGHT = 1e-07
SIGMA_RGB = 15.0
SIGMA_XY_EFF = 50.0  # SIGMA_XY * SCALE
IGNORE_LABEL = 255

N_IMG = 4
K_CLS = 21
H_DS = 64
P = H_DS * H_DS  # 4096
R_EIG = 4
R2 = R_EIG * R_EIG  # 16
N_MONO = 4  # psi = (1, r, g, b)
NBLK = 32  # pixel blocks of 128
AUG_BLKS = 8  # blocks per aug op
W_AUG = N_MONO * K_CLS  # 84
W_AUG3 = (N_MONO - 1) * K_CLS  # 63 (psi_1..3 part)
C_BLK = K_CLS + N_MONO  # 25: per-block [ab | psi] interleave
C_INAB = NBLK * C_BLK  # 800

BF16 = ml_dtypes.bfloat16

_CACHE = {}


def _build_program():
    import concourse.bacc as bacc
    import concourse.tile as tile
    from concourse import mybir

    f32 = mybir.dt.float32
    bf16 = mybir.dt.bfloat16

    nc = bacc.Bacc("TRN2", target_bir_lowering=False, debug=False)

    inab_d = nc.dram_tensor("inab", [128, C_INAB], bf16, kind="ExternalInput")
    kc_d = nc.dram_tensor("kc", [128, NBLK * R2], bf16, kind="ExternalInput")
    ua_d = nc.dram_tensor("u_a", [R2, W_AUG], f32, kind="ExternalOutput")
    ub_d = nc.dram_tensor("u_b", [R2, W_AUG], f32, kind="ExternalOutput")

    with tile.TileContext(nc) as tc:
        with (
            tc.tile_pool(name="const", bufs=1) as cpool,
            tc.tile_pool(name="aug", bufs=4) as augpool,
            tc.tile_pool(name="ps", bufs=1, space="PSUM") as pspool,
            tc.tile_pool(name="outp", bufs=1) as opool,
        ):
            inab = cpool.tile([128, C_INAB], bf16, tag="inab")
            kc = cpool.tile([128, NBLK * R2], bf16, tag="kc")
            half_ab = 20 * C_BLK  # first 20 blocks; rest arrive mid-stream
            half_kc = NBLK * R2 // 2
            # Per-DMA completion latency is ~2us regardless of size, so keep
            # the critical first chunk as a single DMA per queue.
            nc.sync.dma_start(inab[:, 0:half_ab], inab_d[:, 0:half_ab])
            nc.scalar.dma_start(kc[:, 0:half_kc], kc_d[:, 0:half_kc])
            nc.sync.dma_start(inab[:, half_ab:], inab_d[:, half_ab:])
            nc.scalar.dma_start(kc[:, half_kc:], kc_d[:, half_kc:])

            osb = opool.tile([128, W_AUG], f32, tag="o")
            ps = pspool.tile([128, W_AUG], f32, tag="ps")

            inab3 = inab[:].rearrange("p (b c) -> p b c", c=C_BLK)

            # The m0 (psi_0 = 1) matmuls depend only on the inputs, not the
            # aug stream — emit them all first per group so the PE's tail
            # after the final aug op is minimal.  start=True must be the
            # group's first write at the tile position (it clears
            # has_written for the whole position); stop=True goes on the
            # group's final aug-MM.
            for grp in range(2):
                for blk in range(grp * 16, grp * 16 + 16):
                    nc.tensor.matmul(
                        ps[32 * grp : 32 * grp + R2, 0:K_CLS],
                        kc[:, blk * R2 : (blk + 1) * R2],
                        inab3[:, blk, 0:K_CLS],
                        start=(blk % 16 == 0),
                        stop=False,
                        tile_position=(0, 32 * grp),
                        skip_group_check=True,
                    )

            # Descending aug-op sizes keep the DVE rate amortized while
            # shrinking the PE tail behind the last op.
            aug_sizes = [8, 8, 8, 4, 2, 2]
            b0 = 0
            for sz in aug_sizes:
                grp = b0 // 16
                aug = augpool.tile([128, sz * W_AUG3], bf16, tag="aug")
                a4 = inab3[:, b0 : b0 + sz, 0:K_CLS]
                p3 = inab3[:, b0 : b0 + sz, K_CLS + 1 : C_BLK]
                o4 = aug[:].rearrange(
                    "p (b m k) -> p b m k", m=N_MONO - 1, k=K_CLS
                )
                nc.vector.tensor_tensor(
                    o4,
                    a4.unsqueeze(2).broadcast_to([128, sz, N_MONO - 1, K_CLS]),
                    p3.unsqueeze(3).broadcast_to([128, sz, N_MONO - 1, K_CLS]),
                    mybir.AluOpType.mult,
                )
                for j in range(sz):
                    blk = b0 + j
                    nc.tensor.matmul(
                        ps[32 * grp : 32 * grp + R2, K_CLS:W_AUG],
                        kc[:, blk * R2 : (blk + 1) * R2],
                        aug[:, j * W_AUG3 : (j + 1) * W_AUG3],
                        start=False,
                        stop=(blk % 16 == 15),
                        tile_position=(0, 32 * grp),
                        skip_group_check=True,
                    )
                b0 += sz
            nc.vector.tensor_copy(osb[0:R2, :], ps[0:R2, :])
            nc.sync.dma_start(ua_d[:], osb[0:R2, :], single_packet=True)
            nc.vector.tensor_copy(osb[32 : 32 + R2, :], ps[32 : 32 + R2, :])
            nc.scalar.dma_start(ub_d[:], osb[32 : 32 + R2, :], single_packet=True)

    nc.compile()
    return nc


def _host_prep(images, segmentations, ROIs, seg_label):
    """Returns the 8 per-core input dicts. Core 2i -> A-side of image i,
    core 2i+1 -> B-side."""
    imgs = images[:, :, ::2, ::2].astype(np.float64)  # [N,3,64,64]
    segs = (
        segmentations.astype(np.float64)
        .reshape(N_IMG, K_CLS, H_DS, 2, H_DS, 2)
        .mean(axis=(3, 5))
    )
    rois = ROIs[:, ::2, ::2].astype(np.float64)
    lbl = seg_label[:, 0, ::2, ::2]
    unlabel = lbl == IGNORE_LABEL

    seg_max = segs.max(axis=1)
    gate = np.where(unlabel, 1.0, rois - seg_max)
    gate = np.maximum(gate, 0.0)  # [N,64,64]
    seg_r = segs * rois[:, None]  # [N,21,64,64]

    yy, xx = np.meshgrid(
        np.arange(H_DS, dtype=np.float64),
        np.arange(H_DS, dtype=np.float64),
        indexing="ij",
    )
    sq_xy = ((xx / SIGMA_XY_EFF) ** 2 + (yy / SIGMA_XY_EFF) ** 2).reshape(P)
    u = imgs.reshape(N_IMG, 3, P) / SIGMA_RGB  # [N,3,P]
    e = np.exp(-0.5 * (sq_xy[None, :] + (u * u).sum(axis=1)))  # [N,P]
    Bp = seg_r.reshape(N_IMG, K_CLS, P) * e[:, None, :]
    Ap = Bp * gate.reshape(N_IMG, P)[:, None, :]

    # constant Kronecker factor K = (Q sqrt(L)) x (Q sqrt(L)), top R_EIG
    ax = np.arange(H_DS, dtype=np.float64) / SIGMA_XY_EFF
    M = np.exp(np.outer(ax, ax))
    w_eig, Q = np.linalg.eigh(M)
    lam = w_eig[::-1][:R_EIG]
    Qr = Q[:, ::-1][:, :R_EIG]
    Ky = Qr * np.sqrt(lam)[None, :]  # [64, R]
    Kfull = np.einsum("yi,xj->yxij", Ky, Ky).reshape(P, R2)

    def blockmajor(x):  # [P, C] f64 -> [128, 32*C] bf16
        c = x.shape[1]
        return np.ascontiguousarray(
            x.reshape(NBLK, 128, c).transpose(1, 0, 2).reshape(128, NBLK * c)
        ).astype(BF16)

    kc_bm = blockmajor(Kfull)

    in_maps = []
    for img in range(N_IMG):
        psi = np.concatenate([np.ones((1, P)), u[img]], axis=0).T  # [P, 4]
        for side_mat in (Ap[img], Bp[img]):  # A side then B side
            inab = np.concatenate([side_mat.T, psi], axis=1)  # [P, 25]
            in_maps.append(
                {"inab": blockmajor(inab), "kc": kc_bm}
            )
    return in_maps


def _get_program():
    if "nc" not in _CACHE:
        _CACHE["nc"] = _build_program()
    return _CACHE["nc"]


def _install_profile_hook():
    """Best-effort registration of the axon NTFF profile hook so that
    trace=True works (used by test harness, not the plain kernel path)."""
    import sys
    import types

    if "antenv.axon_hooks" in sys.modules:
        return
    try:
        from trn_agent_boot.trn_boot import _ntff_profile_via_ctypes

        hook = _ntff_profile_via_ctypes("/opt/axon/libaxon_pjrt.so")
        mod = types.ModuleType("antenv.axon_hooks")
        mod.get_axon_ntff_profile_hook = lambda: hook
        sys.modules["antenv.axon_hooks"] = mod
    except Exception:
        pass


def kernel(images, segmentations, ROIs, seg_label, _trace=False, _tmpdir=None):
    from concourse import bass_utils

    in_maps = _host_prep(images, segmentations, ROIs, seg_label)
    nc = _get_program()
    if _trace:
        _install_profile_hook()
        bass_utils.upload_artifacts = lambda tmpdir: f"local:{tmpdir}"
    res = bass_utils.run_bass_kernel_spmd(
        nc, in_maps, list(range(8)), trace=_trace, tmpdir=_tmpdir
    )
    total = 0.0
    us = []
    for r in res.results:
        us.append(
            r["u_a"].astype(np.float64) + r["u_b"].astype(np.float64)
        )  # [16, 84]
    for img in range(N_IMG):
        total += np.sum(us[2 * img] * us[2 * img + 1])
    loss = np.float32(-WEIGHT / N_IMG * total)
    if _trace:
        return np.array([loss], np.float32), res
    return np.array([loss], np.float32)
